# revision 21
# baseline (speedup 1.0000x reference)
"""MoE FFN (D=1024, F=4096, E=4, top-2) Trainium2 Bass kernel.

Strategy: expert-parallel dispatch. The router (a 8192x1024x4 matmul +
softmax + top-2) is computed on host in float64 -- it is 0.01% of the
model FLOPs and its only role is to pick the token->expert assignment
that defines the sharding.  Each expert is served by 2 of the 8 cores;
the host gathers each core's assigned tokens into a transposed
[D, C] activation block (capacity C, zero-padded), so the device kernel
is a dense single-expert FFN with tokens on the free dimension:

  h[F, C]  = gelu(W1^T @ xT + b1)        (bf16 matmuls, ACT gelu)
  oT[D, C] = g * (W2^T @ h + b2)         (bf16 matmuls, fp32 psum/acc)

The host then scatter-adds the two gated expert outputs per token.
Only the 2 selected experts per token are ever computed (2x fewer
matmul FLOPs than the dense reference), weights are read from HBM
exactly once, and there are no on-device transposes.

bf16 runs the PE at 1 cycle/row (same as fp32r) but halves LDWEIGHTS
time (stationary reload is the per-matmul overhead) and all weight DMA.
PSUM accumulation stays fp32; only matmul operand storage is bf16.

DMA queues are specialized so nothing blocks the critical path:
gpsimd = weight slabs (one contiguous DMA per 128-partition slab),
sync = xT chunks (chunk-major, so compute starts after ~1MB) then the
slow stride-0 gate broadcast (needed only in the last phase) then
outputs; scalar = the two small host-pretransposed bias tiles, before
any gelu work arrives on that engine.
"""
import math
import numpy as np
import ml_dtypes
from contextlib import ExitStack

try:
    from scipy.special import erf as _erf
except ImportError:              # tiny overflow batches: exact but slower
    _erf = np.vectorize(math.erf)

import concourse.bass as bass
import concourse.tile as tile
from concourse import mybir, bacc
from concourse.bass_utils import run_bass_kernel_spmd

DT = mybir.dt
AFT = mybir.ActivationFunctionType
ALU = mybir.AluOpType

N_CORES = 8
B, S, D, F, E = 4, 2048, 1024, 4096, 4
TOKENS = B * S                  # 8192 tokens, 16384 (token, expert) jobs
TOP_K = 2
P = 128
DC = D // P                     # 8 d-chunks
FC = F // P                     # 32 f-chunks
NCC = 4                         # token chunks per core (each exactly 512:
                                # full psum bank, best matmul overhead amortization)
C_DEFAULT = 2048                # per-core device capacity; seed-0 max load is
                                # 2101, the ~0.9% overflow jobs are computed
                                # exactly on host (capacity-factor pattern
                                # with exact overflow instead of dropping)
NPH = 4                         # F phases; FPH f-chunks of h live at a time
FPH = FC // NPH                 # 8
MM_DT = DT.bfloat16
NP_MM = ml_dtypes.bfloat16

_CACHE = {}


def _ffn(tc, xT, w1, b1, w2, b2, g, out, C):
    nc = tc.nc
    CC = C // NCC
    FW = FPH * P                # 1024 f columns per W1 phase slab
    with ExitStack() as ctx:
        singles = ctx.enter_context(tc.tile_pool(name="singles", bufs=1))
        b1_sb = singles.tile([P, FC], DT.float32)
        b2_sb = singles.tile([P, DC], DT.float32)
        G = singles.tile([P, C], DT.float32)

        # resident activations: xT (input, bf16) and acc (fp32 partials)
        xp = ctx.enter_context(tc.tile_pool(name="xp", bufs=1))
        # xT in chunk-PAIR tiles [P, 2*CC]: halves the DMA issue count and
        # lands cc1 with cc0, removing the phase-0 just-in-time hiccups
        xts = [[xp.tile([P, 2 * CC], MM_DT, name=f"x{d}_{p}")
                for p in range(NCC // 2)] for d in range(DC)]
        accp = ctx.enter_context(tc.tile_pool(name="acc", bufs=1))
        acc = [accp.tile([P, C], DT.float32, name=f"acc{d}") for d in range(DC)]

        # biases host-pretransposed: fast DMAs on the scalar queue, done
        # before the first gelu needs b1 and without delaying gelu issue.
        # xT streams chunk-major on sync so the first chain starts ~1MB in.
        nc.scalar.dma_start(b1_sb[:], b1[:, :])
        nc.scalar.dma_start(b2_sb[:], b2[:, :])
        for p in range(NCC // 2):
            for d in range(DC):
                nc.sync.dma_start(xts[d][p][:],
                                  xT[d * P:(d + 1) * P,
                                     2 * p * CC:2 * (p + 1) * CC])
        # gate broadcast is slow (stride-0, 128x read amplification) but only
        # needed in the last phase: park it behind xT on the sync queue.
        nc.sync.dma_start(G[:], bass.AP(tensor=g.tensor, offset=0,
                                        ap=[[0, P], [1, C]]))

        hp = ctx.enter_context(tc.tile_pool(name="hp", bufs=1))
        h = [hp.tile([P, C], MM_DT, name=f"h{f}") for f in range(FPH)]
        # batched weight slabs: W1 [128d x 1024f] per (phase, d);
        # W2 [128f x 1024d(=D)] per f-chunk.  One contiguous DMA each.
        w1p = ctx.enter_context(tc.tile_pool(name="w1p", bufs=10))
        w2p = ctx.enter_context(tc.tile_pool(name="w2p", bufs=10))
        ps1 = ctx.enter_context(tc.tile_pool(name="ps1", bufs=4, space="PSUM"))
        ps2 = ctx.enter_context(tc.tile_pool(name="ps2", bufs=4, space="PSUM"))
        op = ctx.enter_context(tc.tile_pool(name="op", bufs=8))

        for ph in range(NPH):
            f0 = ph * FPH
            # ---- W1 slab: h[fi] = gelu(W1[:, slab]^T xT + b1) ----
            w1t = [w1p.tile([P, FW], MM_DT, name="w1t") for _ in range(DC)]
            for d in range(DC):
                nc.gpsimd.dma_start(w1t[d][:],
                                    w1[d * P:(d + 1) * P, f0 * P:f0 * P + FW])
            w2t = [w2p.tile([P, D], MM_DT, name="w2t") for _ in range(FPH)]
            for fi in range(FPH):
                fg = f0 + fi
                nc.gpsimd.dma_start(w2t[fi][:], w2[fg * P:(fg + 1) * P, :])
            for c in range(NCC):
                for fi in range(FPH):
                    pt = ps1.tile([P, CC], DT.float32, name="pt")
                    for d in range(DC):
                        nc.tensor.matmul(pt[:],
                                         w1t[d][:, fi * P:(fi + 1) * P],
                                         xts[d][c // 2][:, (c % 2) * CC:
                                                        (c % 2 + 1) * CC],
                                         start=(d == 0), stop=(d == DC - 1))
                    nc.scalar.activation(h[fi][:, c * CC:(c + 1) * CC], pt[:],
                                         AFT.Gelu, bias=b1_sb[:, f0 + fi:f0 + fi + 1],
                                         scale=1.0)
            # ---- W2 slab: acc[d] += W2[slab, :]^T h ----
            for d in range(DC):
                for c in range(NCC):
                    pt = ps2.tile([P, CC], DT.float32, name="pt2")
                    for fi in range(FPH):
                        nc.tensor.matmul(pt[:],
                                         w2t[fi][:, d * P:(d + 1) * P],
                                         h[fi][:, c * CC:(c + 1) * CC],
                                         start=(fi == 0), stop=(fi == FPH - 1))
                    csl = slice(c * CC, (c + 1) * CC)
                    if ph == 0:
                        # seed acc with b2 while copying out of psum
                        nc.scalar.activation(acc[d][:, csl], pt[:], AFT.Identity,
                                             bias=b2_sb[:, d:d + 1], scale=1.0)
                    elif ph < NPH - 1:
                        nc.vector.tensor_add(acc[d][:, csl], acc[d][:, csl], pt[:])
                    else:
                        t = op.tile([P, CC], DT.float32, name="ot")
                        nc.vector.tensor_add(t[:], acc[d][:, csl], pt[:])
                        nc.vector.tensor_mul(t[:], t[:], G[:, csl])
                        nc.sync.dma_start(out[d * P:(d + 1) * P, csl], t[:])


def _build(C):
    nc = bacc.Bacc("TRN2", target_bir_lowering=False, debug=False,
                   num_devices=N_CORES)
    xT = nc.dram_tensor("xt", [D, C], MM_DT, kind="ExternalInput").ap()
    w1 = nc.dram_tensor("w1", [D, F], MM_DT, kind="ExternalInput").ap()
    b1 = nc.dram_tensor("b1", [P, FC], DT.float32, kind="ExternalInput").ap()
    w2 = nc.dram_tensor("w2", [F, D], MM_DT, kind="ExternalInput").ap()
    b2 = nc.dram_tensor("b2", [P, DC], DT.float32, kind="ExternalInput").ap()
    g = nc.dram_tensor("g", [C], DT.float32, kind="ExternalInput").ap()
    out = nc.dram_tensor("out", [D, C], DT.float32, kind="ExternalOutput").ap()
    with tile.TileContext(nc) as tc:
        _ffn(tc, xT, w1, b1, w2, b2, g, out, C)
    nc.finalize()
    return nc


def get_nc(C=C_DEFAULT):
    if C not in _CACHE:
        _CACHE[C] = _build(C)
    return _CACHE[C]


def route(x, Wr, br):
    """Host router in float64: top-2 expert ids + renormalized gates.

    The rank2/rank3 prob gap is >=2.8e-5 on this data, so any router
    accurate to ~1e-6 (f64 trivially is) selects the same experts as the
    f32 reference; gate values agree to ~3e-6.
    """
    xf = x.reshape(TOKENS, D).astype(np.float64)
    logits = xf @ Wr.astype(np.float64) + br.astype(np.float64)
    m = logits.max(axis=-1, keepdims=True)
    ez = np.exp(logits - m)
    probs = ez / ez.sum(axis=-1, keepdims=True)
    order = np.argsort(-probs, axis=-1, kind="stable")
    top2 = order[:, :TOP_K]
    p2 = np.take_along_axis(probs, top2, axis=1)
    gates = (p2 / p2.sum(axis=-1, keepdims=True)).astype(np.float32)
    return top2, gates


def dispatch(x, Wr, br):
    """Token->core assignment: expert e is served by cores 2e and 2e+1."""
    top2, gates = route(x, Wr, br)
    toks, gvals = [], []
    for e in range(E):
        hit = top2 == e                        # (TOKENS, 2)
        te = np.nonzero(hit.any(axis=1))[0]
        ge = (gates * hit).sum(axis=1)[te].astype(np.float32)
        n = len(te)
        half = (n + 1) // 2
        toks.extend([te[:half], te[half:]])
        gvals.extend([ge[:half], ge[half:]])
    return toks, gvals


def make_in_maps(inputs, C=C_DEFAULT):
    x = np.ascontiguousarray(np.asarray(inputs["x"], dtype=np.float32))
    Wr = np.asarray(inputs["Wr"], dtype=np.float32)
    br = np.asarray(inputs["br"], dtype=np.float32)
    W1 = np.asarray(inputs["W1"], dtype=np.float32)
    b1 = np.ascontiguousarray(np.asarray(inputs["b1"], dtype=np.float32))
    W2 = np.asarray(inputs["W2"], dtype=np.float32)
    b2 = np.ascontiguousarray(np.asarray(inputs["b2"], dtype=np.float32))

    toks, gvals = dispatch(x, Wr, br)

    xTfull = x.reshape(TOKENS, D).T.astype(NP_MM)  # [D, TOKENS] contiguous
    w1b = [np.ascontiguousarray(W1[e]).astype(NP_MM) for e in range(E)]
    w2b = [np.ascontiguousarray(W2[e]).astype(NP_MM) for e in range(E)]
    in_maps = []
    for cid in range(N_CORES):
        e = cid // 2
        cnt = min(len(toks[cid]), C)
        xt_c = np.zeros((D, C), dtype=NP_MM)
        xt_c[:, :cnt] = xTfull[:, toks[cid][:cnt]]
        g_c = np.zeros((C,), dtype=np.float32)
        g_c[:cnt] = gvals[cid][:cnt]
        in_maps.append({"xt": xt_c, "w1": w1b[e],
                        "b1": np.ascontiguousarray(b1[e].reshape(FC, P).T),
                        "w2": w2b[e],
                        "b2": np.ascontiguousarray(b2[e].reshape(DC, P).T),
                        "g": g_c})
    return in_maps, (toks, gvals), C


def kernel(x, Wr, br, W1, b1, W2, b2):
    inputs = {"x": x, "Wr": Wr, "br": br, "W1": W1, "b1": b1,
              "W2": W2, "b2": b2}
    in_maps, (toks, gvals), C = make_in_maps(inputs)
    nc = get_nc(C)
    res = run_bass_kernel_spmd(nc, in_maps, core_ids=list(range(N_CORES)))
    outT = np.zeros((D, TOKENS), dtype=np.float32)
    for cid in range(N_CORES):
        cnt = min(len(toks[cid]), C)
        outT[:, toks[cid][:cnt]] += res.results[cid]["out"][:, :cnt]
    # jobs beyond the device capacity (~0.9% on this data) are computed
    # exactly on host in f32 and combined the same way
    xf = np.asarray(inputs["x"], dtype=np.float32).reshape(TOKENS, D)
    W1f = np.asarray(W1, dtype=np.float32)
    W2f = np.asarray(W2, dtype=np.float32)
    b1f = np.asarray(b1, dtype=np.float32)
    b2f = np.asarray(b2, dtype=np.float32)
    for cid in range(N_CORES):
        if len(toks[cid]) > C:
            e = cid // 2
            ot, og = toks[cid][C:], gvals[cid][C:]
            hv = xf[ot] @ W1f[e] + b1f[e]
            hv = 0.5 * hv * (1.0 + _erf(hv / np.sqrt(2.0)))
            oo = hv @ W2f[e] + b2f[e]
            outT[:, ot] += (og[:, None] * oo).T
    return np.ascontiguousarray(outT.T).reshape(B, S, D)
